# revision 5
# baseline (speedup 1.0000x reference)
"""Trainium2 Bass kernel for the seasonal-decomposition block.

Math: for each season s, circ_s = real(F_s^H diag(d_s) F_s) with F_s the s-th
diagonal LxL block of the normalized N=L*S DFT matrix. Expanding,
    circ_s[a, b] = (1/N) * sum_j d_s[j] * cos(2*pi*(s*L+j)*(a-b)/N)
depends only on a-b: a symmetric Toeplitz matrix whose first column
c_s(t) is computed on host with one length-N FFT. Every 128x128 block of
circ_s is a column slice of the skewed buffer
    E2[p, m] = c_s(|1920 + p - m|)   (shape [128, 3968])
so the LxL matrix is never materialized.

The recurrence  x_rem <- x_rem - tanh(x_rem @ circ_s)  runs in transposed
layout (positions on partitions, rows on the free axis). Matmuls use fp8
(e4m3) operands in DoubleRow perf mode: each instruction contracts TWO
128-position chunks (lhsT [128,2,128], rhs [128,2,RPC]) at 0.5 cycles/row,
2-4x the fp32r rate. In the skewed buffer adjacent-diagonal block pairs
(d+1, d) are contiguous, which matches DoubleRow's k-tile layout when the
moving x chunks are stored pair-swapped (slice 0 = odd chunk, slice 1 =
even chunk). Weights carry a per-season power-of-2 scale (max|c|*S <= 224
< e4m3 max 240); the inverse is folded into the tanh activation's input
scale. Accuracy: fp8 noise on both operands gives ~1.2e-2 max rel err
(tolerance 2e-2, verified in numpy simulation).

State precision: corr[b] accumulates sum_s tanh_s in fp32 (this is also the
output term), and x_rem is re-derived as x(bf16) - corr for each season's
fp8 matmul operand, so no fp32 x_rem tiles are carried. Engines per chunk:
PE 8 DoubleRow matmuls, ACT tanh (scaled), Pool corr accumulate, DVE the
fp8 subtract. The trailing avg-pool trend is two f32r banded matmuls per
128-chunk on xr4 = x - corr tiles.

Sharding: pure data-parallel over the B*C = 2048 rows, 256 rows per core,
8 cores, no collectives.
"""

import sys

sys.path.insert(0, "/opt/trn_rl_repo")

import ml_dtypes
import numpy as np

import concourse.mybir as mybir
import concourse.tile as tile
from concourse import bacc
from concourse.bass_utils import run_bass_kernel_spmd

L = 2048
S = 4
NFULL = L * S
KER = 25
B, C = 64, 32
NCORES = 8
ROWS = B * C          # 2048
RPC = ROWS // NCORES  # 256 rows per core
NCHUNK = L // 128     # 16
NPAIR = NCHUNK // 2   # 8

_f32 = mybir.dt.float32
_f32r = mybir.dt.float32r
_f8 = mybir.dt.float8e4
_bf16 = mybir.dt.bfloat16
_np_f8 = ml_dtypes.float8_e4m3
_np_bf16 = ml_dtypes.bfloat16
_DR = mybir.MatmulPerfMode.DoubleRow


def _build_tband():
    """Three [128,128] band blocks of the avg-pool matrix T (trend = T.T @ x)."""
    u = np.arange(128)[:, None]
    t = np.arange(128)[None, :]
    diag = ((t - u >= 0) & (t - u <= KER - 1)).astype(np.float32) / KER
    sub = ((u - t) >= 128 - (KER - 1)).astype(np.float32) / KER
    t00 = diag.copy()
    t00[0, :] += np.maximum(0, (KER - 1) - np.arange(128)).astype(np.float32) / KER
    return np.ascontiguousarray(np.stack([t00, diag, sub], axis=1))  # [128, 3, 128]


_TBAND = _build_tband()
# Skewed-buffer gather index: E2[p, m] = c(|1920 + p - m|), m in [0, 3968).
# Block pair for (output chunk b, input pair p) sits at k-slice 14 - 2p + b
# of the [128, 31, 128] view: slice k = diag block d+1, slice k+1 = block d,
# d = 2p - b, matching the pair-swapped moving operand.
_E2_IDX = np.abs(1920 + np.arange(128)[:, None] - np.arange(31 * 128)[None, :])


def _circ_cols(diagonals):
    """First columns c_s(t), t = 0..L-1, of each season's Toeplitz circ_s."""
    d = np.zeros((S, NFULL), dtype=np.float64)
    d[:, :L] = np.asarray(diagonals, dtype=np.float64)
    F = np.fft.fft(d, axis=1)  # F[s,k] = sum_j d_j e^{-2pi i jk/N}
    t = np.arange(L)
    ph = np.exp((2j * np.pi / NFULL) * (np.arange(S)[:, None] * L * t[None, :]))
    return ((ph * np.conj(F[:, :L])).real / NFULL).astype(np.float32)  # [S, L]


def _season_scales(c):
    """Per-season power-of-2 scale keeping max|c*S| <= 224 (e4m3 max 240)."""
    return tuple(
        float(2.0 ** np.floor(np.log2(224.0 / max(np.abs(c[s]).max(), 1e-30))))
        for s in range(S)
    )


def _emit_body(nc, pools, xp8_d, xbf_d, e2_d, tb_d, out_d, scales):
    constp, xrp, corrp, workp, psum_a, psum_t = pools
    tanh_f = mybir.ActivationFunctionType.Tanh

    e2_sb = [constp.tile([128, 31, 128], _f8, tag=f"e2_{s}", name=f"e2_{s}") for s in range(S)]
    xbf_sb = constp.tile([128, NCHUNK, RPC], _bf16, tag="xbf")
    tb_sb = constp.tile([128, 3, 128], _f32r, tag="tb")

    # Prologue DMA order follows first use: season-0 chains start at the low
    # k-slices of e2[0] and consume all x pairs.
    nc.sync.dma_start(e2_sb[0][:, :16, :], e2_d[0][:, :16, :])
    xr_cur = []
    for p in range(NPAIR):
        t = xrp.tile([128, 2, RPC], _f8, tag=f"xrp{p}", name=f"x0_{p}")
        nc.sync.dma_start(t[:], xp8_d[p])
        xr_cur.append(t)
    nc.sync.dma_start(e2_sb[0][:, 16:, :], e2_d[0][:, 16:, :])
    nc.sync.dma_start(xbf_sb[:], xbf_d[:])
    for s in range(1, S):
        nc.sync.dma_start(e2_sb[s][:], e2_d[s])
    nc.sync.dma_start(tb_sb[:], tb_d[:])

    corr = [corrp.tile([128, RPC], _f32, tag=f"corr{b}", name=f"corr{b}") for b in range(NCHUNK)]
    big_ob = constp.tile([128, NCHUNK, RPC], _f32, tag="bigob")
    xr4 = [None] * NCHUNK

    def emit_trend(j):
        tps = psum_t.tile([128, RPC], _f32, tag="acc" if psum_t is psum_a else "tps", name=f"tps{j}")
        if j == 0:
            nc.tensor.matmul(tps[:], tb_sb[:, 0, :], xr4[0][:], start=True, stop=True)
        else:
            nc.tensor.matmul(tps[:], tb_sb[:, 2, :], xr4[j - 1][:], start=True, stop=False)
            nc.tensor.matmul(tps[:], tb_sb[:, 1, :], xr4[j][:], start=False, stop=True)
        nc.vector.tensor_add(out=big_ob[:, j, :], in0=corr[j][:], in1=tps[:])
        if j % 4 == 3:
            q = j // 4
            nc.sync.dma_start(out_d[:, 4 * q : 4 * q + 4, :], big_ob[:, 4 * q : 4 * q + 4, :])

    for s in range(S):
        inv = 1.0 / scales[s]
        xr_next = None
        if s < S - 1:
            xr_next = [
                xrp.tile([128, 2, RPC], _f8, tag=f"xrp{p}", name=f"xr{s + 1}_{p}")
                for p in range(NPAIR)
            ]
        for b in range(NCHUNK):
            acc = psum_a.tile([128, RPC], _f32, tag="acc")
            # Pair order 0..7 matches xr production order from the previous
            # season (chunk pairs complete in order), minimizing the
            # season-boundary bubble.
            for p in range(NPAIR):
                k = 14 - 2 * p + b
                nc.tensor.matmul(
                    acc[:],
                    e2_sb[s][:, k : k + 2, :],
                    xr_cur[p][:],
                    start=(p == 0),
                    stop=(p == NPAIR - 1),
                    perf_mode=_DR,
                )
            if s == 0:
                nc.scalar.activation(corr[b][:], acc[:], tanh_f, scale=inv)
            else:
                tmp = workp.tile([128, RPC], _f32, tag="tanh")
                nc.scalar.activation(tmp[:], acc[:], tanh_f, scale=inv)
                nc.gpsimd.tensor_add(out=corr[b][:], in0=corr[b][:], in1=tmp[:])
            if s < S - 1:
                nc.vector.tensor_sub(
                    out=xr_next[b // 2][:, 1 - (b % 2), :],
                    in0=xbf_sb[:, b, :],
                    in1=corr[b][:],
                )
            else:
                t4 = workp.tile([128, RPC], _f32r, tag="xr4", name=f"xr4_{b}")
                xr4[b] = t4
                nc.vector.tensor_sub(out=t4[:], in0=xbf_sb[:, b, :], in1=corr[b][:])
                # Interleave trend chunks two groups behind so the PE never
                # waits on the DVE subs they read.
                if b >= 2:
                    emit_trend(b - 2)
        if xr_next is not None:
            xr_cur = xr_next

    emit_trend(NCHUNK - 2)
    emit_trend(NCHUNK - 1)


def build_nc(scales, reps=1, acc_bufs=8, merge_tps=True):
    nc = bacc.Bacc("TRN2", target_bir_lowering=False, debug=False)
    xp8_d = nc.dram_tensor("xp8", [NPAIR, 128, 2, RPC], _f8, kind="ExternalInput")
    xbf_d = nc.dram_tensor("xbf", [128, NCHUNK, RPC], _bf16, kind="ExternalInput")
    e2_d = nc.dram_tensor("e2", [S, 128, 31, 128], _f8, kind="ExternalInput")
    tb_d = nc.dram_tensor("tb", [128, 3, 128], _f32r, kind="ExternalInput")
    out_d = nc.dram_tensor("out", [128, NCHUNK, RPC], _f32, kind="ExternalOutput")

    with tile.TileContext(nc) as tc:
        with (
            tc.tile_pool(name="const", bufs=1) as constp,
            tc.tile_pool(name="xrp", bufs=2) as xrp,
            tc.tile_pool(name="corrp", bufs=1) as corrp,
            tc.tile_pool(name="work", bufs=4) as workp,
            tc.tile_pool(name="psum_a", bufs=acc_bufs, space="PSUM") as psum_a,
            tc.tile_pool(name="psum_t", bufs=2, space="PSUM") as psum_t,
        ):
            pools = (constp, xrp, corrp, workp, psum_a,
                     psum_a if merge_tps else psum_t)
            if reps == 1:
                _emit_body(nc, pools, xp8_d, xbf_d, e2_d, tb_d, out_d, scales)
            else:
                with tc.For_i(0, reps, 1, staggered_reset=True,
                              hint_engines=(mybir.EngineType.PE,)):
                    _emit_body(nc, pools, xp8_d, xbf_d, e2_d, tb_d, out_d, scales)
    nc.compile()
    return nc


_NC_CACHE = {}


def _get_nc(scales, reps=1):
    key = (scales, reps)
    if key not in _NC_CACHE:
        _NC_CACHE[key] = build_nc(scales, reps)
    return _NC_CACHE[key]


def make_in_maps(x, diagonals):
    c = _circ_cols(diagonals)
    scales = _season_scales(c)
    cq = np.stack([c[s] * scales[s] for s in range(S)]).astype(_np_f8)  # [S, L]
    e2 = cq.view(np.uint8)[:, _E2_IDX].view(_np_f8)  # [S, 128, 3968]
    e2 = np.ascontiguousarray(e2.reshape(S, 128, 31, 128))
    xT = np.asarray(x, dtype=np.float32).reshape(ROWS, L).T  # [L, ROWS] view
    in_maps = []
    for i in range(NCORES):
        xs = np.ascontiguousarray(xT[:, i * RPC : (i + 1) * RPC])
        xs = np.ascontiguousarray(
            xs.reshape(NCHUNK, 128, RPC).transpose(1, 0, 2)
        )  # [128, 16, RPC]
        x8 = xs.astype(_np_f8)
        xp8 = np.ascontiguousarray(
            np.stack(
                [np.stack([x8[:, 2 * p + 1, :], x8[:, 2 * p, :]], axis=1) for p in range(NPAIR)]
            )
        )  # [NPAIR, 128, 2, RPC], slice 0 = odd chunk
        in_maps.append(
            {
                "xp8": xp8,
                "xbf": xs.astype(_np_bf16),
                "e2": e2,
                "tb": _TBAND.astype(np.float32),
            }
        )
    return in_maps, scales


def gather_out(results):
    parts = []
    for r in results:
        o = r["out"]  # [128, NCHUNK, RPC]
        parts.append(np.ascontiguousarray(o.transpose(1, 0, 2)).reshape(L, RPC))
    outT = np.concatenate(parts, axis=1)  # [L, ROWS]
    return np.ascontiguousarray(outT.T).reshape(B, C, L).astype(np.float32)


def kernel(x, diagonals):
    x = np.asarray(x, dtype=np.float32)
    assert x.shape == (B, C, L) and np.asarray(diagonals).shape == (S, L)
    in_maps, scales = make_in_maps(x, diagonals)
    nc = _get_nc(scales, 1)
    last_err = None
    for attempt in range(3):
        try:
            res = run_bass_kernel_spmd(nc, in_maps, core_ids=list(range(NCORES)))
            return gather_out(res.results)
        except Exception as ex:  # transient device errors (e.g. NRT_EXEC_UNIT_UNRECOVERABLE)
            last_err = ex
            import time as _time

            _time.sleep(2.0 * (attempt + 1))
    raise last_err


# revision 6
# speedup vs baseline: 1.1278x; 1.1278x over previous
"""Trainium2 Bass kernel for the seasonal-decomposition block.

Math: for each season s, circ_s = real(F_s^H diag(d_s) F_s) with F_s the s-th
diagonal LxL block of the normalized N=L*S DFT matrix. Expanding,
    circ_s[a, b] = (1/N) * sum_j d_s[j] * cos(2*pi*(s*L+j)*(a-b)/N)
depends only on a-b: a symmetric Toeplitz matrix whose first column
c_s(t) is computed on host with one length-N FFT. Every 128x128 block of
circ_s is a column slice of the skewed buffer
    E2[p, m] = c_s(|1920 + p - m|)   (shape [128, 3968])
so the LxL matrix is never materialized.

The recurrence  x_rem <- x_rem - tanh(x_rem @ circ_s)  runs in transposed
layout (positions on partitions, rows on the free axis). Matmuls use fp8
(e4m3) operands in DoubleRow perf mode: each instruction contracts TWO
128-position chunks (lhsT [128,2,128], rhs [128,2,RPC]) at 0.5 cycles/row,
2-4x the fp32r rate. In the skewed buffer adjacent-diagonal block pairs
(d+1, d) are contiguous, which matches DoubleRow's k-tile layout when the
moving x chunks are stored pair-swapped (slice 0 = odd chunk, slice 1 =
even chunk). Weights carry a per-season power-of-2 scale (max|c|*S <= 224
< e4m3 max 240); the inverse is folded into the tanh activation's input
scale. Accuracy: fp8 noise on both operands gives ~1.2e-2 max rel err
(tolerance 2e-2, verified in numpy simulation).

State precision: corr[b] accumulates sum_s tanh_s in fp32 (this is also the
output term), and x_rem is re-derived as x(bf16) - corr for each season's
fp8 matmul operand, so no fp32 x_rem tiles are carried. Engines per chunk:
PE 8 DoubleRow matmuls, ACT tanh (scaled), Pool corr accumulate, DVE the
fp8 subtract. The trailing avg-pool trend is two f32r banded matmuls per
128-chunk on xr4 = x - corr tiles.

Sharding: pure data-parallel over the B*C = 2048 rows, 256 rows per core,
8 cores, no collectives.
"""

import sys

sys.path.insert(0, "/opt/trn_rl_repo")

import ml_dtypes
import numpy as np

import concourse.mybir as mybir
import concourse.tile as tile
from concourse import bacc
from concourse.bass_utils import run_bass_kernel_spmd

L = 2048
S = 4
NFULL = L * S
KER = 25
B, C = 64, 32
NCORES = 8
ROWS = B * C          # 2048
RPC = ROWS // NCORES  # 256 rows per core
NCHUNK = L // 128     # 16
NPAIR = NCHUNK // 2   # 8

_f32 = mybir.dt.float32
_f32r = mybir.dt.float32r
_f8 = mybir.dt.float8e4
_bf16 = mybir.dt.bfloat16
_np_f8 = ml_dtypes.float8_e4m3
_np_bf16 = ml_dtypes.bfloat16
_DR = mybir.MatmulPerfMode.DoubleRow


def _build_tband():
    """Three [128,128] band blocks of the avg-pool matrix T (trend = T.T @ x)."""
    u = np.arange(128)[:, None]
    t = np.arange(128)[None, :]
    diag = ((t - u >= 0) & (t - u <= KER - 1)).astype(np.float32) / KER
    sub = ((u - t) >= 128 - (KER - 1)).astype(np.float32) / KER
    t00 = diag.copy()
    t00[0, :] += np.maximum(0, (KER - 1) - np.arange(128)).astype(np.float32) / KER
    return np.ascontiguousarray(np.stack([t00, diag, sub], axis=1))  # [128, 3, 128]


_TBAND = _build_tband()
# Skewed-buffer gather index: E2[p, m] = c(|1920 + p - m|), m in [0, 3968).
# Block pair for (output chunk b, input pair p) sits at k-slice 14 - 2p + b
# of the [128, 31, 128] view: slice k = diag block d+1, slice k+1 = block d,
# d = 2p - b, matching the pair-swapped moving operand.
_E2_IDX = np.abs(1920 + np.arange(128)[:, None] - np.arange(31 * 128)[None, :])


def _circ_cols(diagonals):
    """First columns c_s(t), t = 0..L-1, of each season's Toeplitz circ_s."""
    d = np.zeros((S, NFULL), dtype=np.float64)
    d[:, :L] = np.asarray(diagonals, dtype=np.float64)
    F = np.fft.fft(d, axis=1)  # F[s,k] = sum_j d_j e^{-2pi i jk/N}
    t = np.arange(L)
    ph = np.exp((2j * np.pi / NFULL) * (np.arange(S)[:, None] * L * t[None, :]))
    return ((ph * np.conj(F[:, :L])).real / NFULL).astype(np.float32)  # [S, L]


def _season_scales(c):
    """Per-season power-of-2 scale keeping max|c*S| <= 224 (e4m3 max 240)."""
    return tuple(
        float(2.0 ** np.floor(np.log2(224.0 / max(np.abs(c[s]).max(), 1e-30))))
        for s in range(S)
    )


def _emit_body(nc, pools, xp8_d, xbf_d, e2_d, tb_d, out_d, scales):
    constp, xrp, corrp, workp, psum_a, psum_t = pools
    tanh_f = mybir.ActivationFunctionType.Tanh

    e2_sb = [constp.tile([128, 31, 128], _f8, tag=f"e2_{s}", name=f"e2_{s}") for s in range(S)]
    xbf_sb = constp.tile([128, NCHUNK, RPC], _bf16, tag="xbf")
    tb_sb = constp.tile([128, 3, 128], _f32r, tag="tb")

    # Prologue DMA order follows first use: season-0 chains start at the low
    # k-slices of e2[0] and consume all x pairs.
    nc.sync.dma_start(e2_sb[0][:, :16, :], e2_d[0][:, :16, :])
    xr_cur = []
    for p in range(NPAIR):
        t = xrp.tile([128, 2, RPC], _f8, tag=f"xrp{p}", name=f"x0_{p}")
        nc.sync.dma_start(t[:], xp8_d[p])
        xr_cur.append(t)
    nc.sync.dma_start(e2_sb[0][:, 16:, :], e2_d[0][:, 16:, :])
    nc.sync.dma_start(xbf_sb[:], xbf_d[:])
    for s in range(1, S):
        nc.sync.dma_start(e2_sb[s][:], e2_d[s])
    nc.sync.dma_start(tb_sb[:], tb_d[:])

    corr = [corrp.tile([128, RPC], _f32, tag=f"corr{b}", name=f"corr{b}") for b in range(NCHUNK)]
    big_ob = constp.tile([128, NCHUNK, RPC], _f32, tag="bigob")
    xr4 = [None] * NCHUNK

    def emit_trend(j):
        tps = psum_t.tile([128, RPC], _f32, tag="acc" if psum_t is psum_a else "tps", name=f"tps{j}")
        if j == 0:
            nc.tensor.matmul(tps[:], tb_sb[:, 0, :], xr4[0][:], start=True, stop=True)
        else:
            nc.tensor.matmul(tps[:], tb_sb[:, 2, :], xr4[j - 1][:], start=True, stop=False)
            nc.tensor.matmul(tps[:], tb_sb[:, 1, :], xr4[j][:], start=False, stop=True)
        nc.vector.tensor_add(out=big_ob[:, j, :], in0=corr[j][:], in1=tps[:])
        if j % 4 == 3:
            q = j // 4
            nc.sync.dma_start(out_d[:, 4 * q : 4 * q + 4, :], big_ob[:, 4 * q : 4 * q + 4, :])

    for s in range(S):
        inv = 1.0 / scales[s]
        xr_next = None
        if s < S - 1:
            xr_next = [
                xrp.tile([128, 2, RPC], _f8, tag=f"xrp{p}", name=f"xr{s + 1}_{p}")
                for p in range(NPAIR)
            ]
        for b in range(NCHUNK):
            acc = psum_a.tile([128, RPC], _f32, tag="acc")
            # Pair order 0..7 matches xr production order from the previous
            # season (chunk pairs complete in order), minimizing the
            # season-boundary bubble.
            for p in range(NPAIR):
                k = 14 - 2 * p + b
                nc.tensor.matmul(
                    acc[:],
                    e2_sb[s][:, k : k + 2, :],
                    xr_cur[p][:],
                    start=(p == 0),
                    stop=(p == NPAIR - 1),
                    perf_mode=_DR,
                )
            if s == 0:
                nc.scalar.activation(corr[b][:], acc[:], tanh_f, scale=inv)
            else:
                tmp = workp.tile([128, RPC], _f32, tag="tanh")
                nc.scalar.activation(tmp[:], acc[:], tanh_f, scale=inv)
                nc.gpsimd.tensor_add(out=corr[b][:], in0=corr[b][:], in1=tmp[:])
            if s < S - 1:
                nc.vector.tensor_sub(
                    out=xr_next[b // 2][:, 1 - (b % 2), :],
                    in0=xbf_sb[:, b, :],
                    in1=corr[b][:],
                )
            else:
                t4 = workp.tile([128, RPC], _f32r, tag="xr4", name=f"xr4_{b}")
                xr4[b] = t4
                nc.vector.tensor_sub(out=t4[:], in0=xbf_sb[:, b, :], in1=corr[b][:])
                # Interleave trend chunks two groups behind so the PE never
                # waits on the DVE subs they read.
                if b >= 2:
                    emit_trend(b - 2)
        if xr_next is not None:
            xr_cur = xr_next

    emit_trend(NCHUNK - 2)
    emit_trend(NCHUNK - 1)


def build_nc(scales, reps=1, acc_bufs=6, merge_tps=True):
    nc = bacc.Bacc("TRN2", target_bir_lowering=False, debug=False)
    xp8_d = nc.dram_tensor("xp8", [NPAIR, 128, 2, RPC], _f8, kind="ExternalInput")
    xbf_d = nc.dram_tensor("xbf", [128, NCHUNK, RPC], _bf16, kind="ExternalInput")
    e2_d = nc.dram_tensor("e2", [S, 128, 31, 128], _f8, kind="ExternalInput")
    tb_d = nc.dram_tensor("tb", [128, 3, 128], _f32r, kind="ExternalInput")
    out_d = nc.dram_tensor("out", [128, NCHUNK, RPC], _f32, kind="ExternalOutput")

    with tile.TileContext(nc) as tc:
        with (
            tc.tile_pool(name="const", bufs=1) as constp,
            tc.tile_pool(name="xrp", bufs=2) as xrp,
            tc.tile_pool(name="corrp", bufs=1) as corrp,
            tc.tile_pool(name="work", bufs=4) as workp,
            tc.tile_pool(name="psum_a", bufs=acc_bufs, space="PSUM") as psum_a,
            tc.tile_pool(name="psum_t", bufs=2, space="PSUM") as psum_t,
        ):
            pools = (constp, xrp, corrp, workp, psum_a,
                     psum_a if merge_tps else psum_t)
            if reps == 1:
                _emit_body(nc, pools, xp8_d, xbf_d, e2_d, tb_d, out_d, scales)
            else:
                with tc.For_i(0, reps, 1, staggered_reset=True,
                              hint_engines=(mybir.EngineType.PE,)):
                    _emit_body(nc, pools, xp8_d, xbf_d, e2_d, tb_d, out_d, scales)
    nc.compile()
    return nc


_NC_CACHE = {}


def _get_nc(scales, reps=1):
    key = (scales, reps)
    if key not in _NC_CACHE:
        _NC_CACHE[key] = build_nc(scales, reps)
    return _NC_CACHE[key]


def make_in_maps(x, diagonals):
    c = _circ_cols(diagonals)
    scales = _season_scales(c)
    cq = np.stack([c[s] * scales[s] for s in range(S)]).astype(_np_f8)  # [S, L]
    e2 = cq.view(np.uint8)[:, _E2_IDX].view(_np_f8)  # [S, 128, 3968]
    e2 = np.ascontiguousarray(e2.reshape(S, 128, 31, 128))
    xT = np.asarray(x, dtype=np.float32).reshape(ROWS, L).T  # [L, ROWS] view
    in_maps = []
    for i in range(NCORES):
        xs = np.ascontiguousarray(xT[:, i * RPC : (i + 1) * RPC])
        xs = np.ascontiguousarray(
            xs.reshape(NCHUNK, 128, RPC).transpose(1, 0, 2)
        )  # [128, 16, RPC]
        x8 = xs.astype(_np_f8)
        xp8 = np.ascontiguousarray(
            np.stack(
                [np.stack([x8[:, 2 * p + 1, :], x8[:, 2 * p, :]], axis=1) for p in range(NPAIR)]
            )
        )  # [NPAIR, 128, 2, RPC], slice 0 = odd chunk
        in_maps.append(
            {
                "xp8": xp8,
                "xbf": xs.astype(_np_bf16),
                "e2": e2,
                "tb": _TBAND.astype(np.float32),
            }
        )
    return in_maps, scales


def gather_out(results):
    parts = []
    for r in results:
        o = r["out"]  # [128, NCHUNK, RPC]
        parts.append(np.ascontiguousarray(o.transpose(1, 0, 2)).reshape(L, RPC))
    outT = np.concatenate(parts, axis=1)  # [L, ROWS]
    return np.ascontiguousarray(outT.T).reshape(B, C, L).astype(np.float32)


def kernel(x, diagonals):
    x = np.asarray(x, dtype=np.float32)
    assert x.shape == (B, C, L) and np.asarray(diagonals).shape == (S, L)
    in_maps, scales = make_in_maps(x, diagonals)
    nc = _get_nc(scales, 1)
    last_err = None
    for attempt in range(3):
        try:
            res = run_bass_kernel_spmd(nc, in_maps, core_ids=list(range(NCORES)))
            return gather_out(res.results)
        except Exception as ex:  # transient device errors (e.g. NRT_EXEC_UNIT_UNRECOVERABLE)
            last_err = ex
            import time as _time

            _time.sleep(2.0 * (attempt + 1))
    raise last_err
